# revision 2
# baseline (speedup 1.0000x reference)
"""Trainium2 Bass kernel for Co-occurrence Infused Multi-Label Attention.

Shards the n_classes (code) axis across 8 NeuronCores. Per core:
  QgT = tanh(trans_wT @ QT)           [tf, c]   (tf on partitions)
  qT  = q_wT @ QgT + q_b              [zh, c]
  QwT = W_wT @ QgT                    [zh, c]
  WKT = tanh(k_wT @ HT + k_b)         [zh, t]   (t = b*512 + token)
  WVT = tanh(v_wT @ HT + v_b)         [zh, t]
  per (c-tile of 128, b, z):
    scores[c, t] = qT_z.T @ WKT_z     (K=64 matmul, row-group packed)
    G[c, t]      = QwT_z.T @ WVT_z    ("G-trick": folds the final per-class
                                       dot with Qw through the t-contraction)
    exp_s, den[c] = ACT Exp with accum_out  (fused denominator)
    R[c] = sum_t exp_s * G            (DVE tensor_tensor_reduce, fused)
  out[c, b] = sum_z R_zb[c] / den_zb[c]
"""

import numpy as np
import ml_dtypes

# Problem constants (hardcoded per harness contract)
C_FULL = 8929
D = 768          # d_model
TF = 512         # transform dim (= NH * DK)
NH = 8           # heads
DK = 64          # head dim
B = 4            # chunks
T = 512          # tokens per chunk
BT = B * T       # 2048
N_CORES = 8
CP = 9216        # padded classes (8 * 1152)
CS = CP // N_CORES   # 1152 classes per core
NDC = D // 128       # 6 d-model chunks
NFC = TF // 128      # 4 transform chunks
C_CHUNKS = [(0, 512), (512, 512), (1024, 128)]  # (offset, width) per core

_BF = ml_dtypes.bfloat16

_CACHE = {}


def _build(a_zero: bool):
    from contextlib import ExitStack
    import concourse.bass as bass
    import concourse.mybir as mybir
    import concourse.tile as tile
    from concourse import bacc

    bf = mybir.dt.bfloat16
    f32 = mybir.dt.float32
    AF = mybir.ActivationFunctionType
    ALU = mybir.AluOpType

    nc = bacc.Bacc()

    qt_d = nc.declare_dram_parameter("qt", [D, CS], bf, isOutput=False)
    ht_d = nc.declare_dram_parameter("ht", [D, BT], bf, isOutput=False)
    wtr_d = nc.declare_dram_parameter("wtr", [D, TF], bf, isOutput=False)
    wq_d = nc.declare_dram_parameter("wq", [TF, TF], bf, isOutput=False)
    wk_d = nc.declare_dram_parameter("wk", [D, TF], bf, isOutput=False)
    wv_d = nc.declare_dram_parameter("wv", [D, TF], bf, isOutput=False)
    ww_d = nc.declare_dram_parameter("ww", [TF, TF], bf, isOutput=False)
    btr_d = nc.declare_dram_parameter("btr", [TF], f32, isOutput=False)
    bq_d = nc.declare_dram_parameter("bq", [TF], f32, isOutput=False)
    bk_d = nc.declare_dram_parameter("bk", [TF], f32, isOutput=False)
    bv_d = nc.declare_dram_parameter("bv", [TF], f32, isOutput=False)
    ea_d = None
    if not a_zero:
        ea_d = nc.declare_dram_parameter("ea", [128, BT], bf, isOutput=False)
    out_d = nc.declare_dram_parameter("out", [CS, B], f32, isOutput=True)

    with tile.TileContext(nc) as tc, ExitStack() as top:
        const = top.enter_context(tc.tile_pool(name="const", bufs=1))

        # --- load weights / H / biases ---
        w_tr = const.tile([128, NDC * TF], bf)
        w_k = const.tile([128, NDC * TF], bf)
        w_v = const.tile([128, NDC * TF], bf)
        for j in range(NDC):
            nc.sync.dma_start(w_tr[:, j * TF:(j + 1) * TF], wtr_d[j * 128:(j + 1) * 128, :])
            nc.sync.dma_start(w_k[:, j * TF:(j + 1) * TF], wk_d[j * 128:(j + 1) * 128, :])
            nc.sync.dma_start(w_v[:, j * TF:(j + 1) * TF], wv_d[j * 128:(j + 1) * 128, :])
        w_q = const.tile([128, NFC * TF], bf)
        w_W = const.tile([128, NFC * TF], bf)
        for j in range(NFC):
            nc.sync.dma_start(w_q[:, j * TF:(j + 1) * TF], wq_d[j * 128:(j + 1) * 128, :])
            nc.sync.dma_start(w_W[:, j * TF:(j + 1) * TF], ww_d[j * 128:(j + 1) * 128, :])
        ht_sb = const.tile([128, NDC * BT], bf)
        for j in range(NDC):
            nc.sync.dma_start(ht_sb[:, j * BT:(j + 1) * BT], ht_d[j * 128:(j + 1) * 128, :])
        b_tr = const.tile([128, NFC], f32)
        b_q = const.tile([128, NFC], f32)
        b_k = const.tile([128, NFC], f32)
        b_v = const.tile([128, NFC], f32)
        nc.sync.dma_start(b_tr[:], btr_d[:].rearrange("(c p) -> p c", p=128))
        nc.sync.dma_start(b_q[:], bq_d[:].rearrange("(c p) -> p c", p=128))
        nc.sync.dma_start(b_k[:], bk_d[:].rearrange("(c p) -> p c", p=128))
        nc.sync.dma_start(b_v[:], bv_d[:].rearrange("(c p) -> p c", p=128))
        ea_sb = None
        if not a_zero:
            ea_sb = const.tile([128, BT], bf)
            nc.sync.dma_start(ea_sb[:], ea_d[:, :])

        # --- K/V transform: WKT/WVT [zh, t] ---
        wkt = const.tile([128, NFC * BT], bf)
        wvt = const.tile([128, NFC * BT], bf)
        with tc.tile_pool(name="kvps", bufs=4, space="PSUM") as kvps:
            for (w_sb, b_sb, dst) in ((w_k, b_k, wkt), (w_v, b_v, wvt)):
                for jz in range(NFC):
                    for jt in range(BT // 512):
                        ps = kvps.tile([128, 512], f32, tag="kv")
                        for jd in range(NDC):
                            nc.tensor.matmul(
                                ps[:],
                                w_sb[:, jd * TF + jz * 128: jd * TF + (jz + 1) * 128],
                                ht_sb[:, jd * BT + jt * 512: jd * BT + (jt + 1) * 512],
                                start=(jd == 0), stop=(jd == NDC - 1))
                        nc.scalar.activation(
                            dst[:, jz * BT + jt * 512: jz * BT + (jt + 1) * 512],
                            ps[:], AF.Tanh, bias=b_sb[:, jz:jz + 1])

        # --- main: per c-chunk Qg chain, per c-tile attention ---
        with ExitStack() as main:
            qin = main.enter_context(tc.tile_pool(name="qin", bufs=2))
            qg = main.enter_context(tc.tile_pool(name="qg", bufs=2))
            chps = main.enter_context(tc.tile_pool(name="chps", bufs=2, space="PSUM"))
            scps = main.enter_context(tc.tile_pool(name="scps", bufs=3, space="PSUM"))
            gps = main.enter_context(tc.tile_pool(name="gps", bufs=3, space="PSUM"))
            expp = main.enter_context(tc.tile_pool(name="expp", bufs=4))
            scr = main.enter_context(tc.tile_pool(name="scr", bufs=4))
            accp = main.enter_context(tc.tile_pool(name="accp", bufs=2))
            outp = main.enter_context(tc.tile_pool(name="outp", bufs=2))

            for (c0, w) in C_CHUNKS:
                qt_sb = qin.tile([128, NDC * 512], bf, tag="qt")
                for jd in range(NDC):
                    nc.sync.dma_start(qt_sb[:, jd * w:(jd + 1) * w],
                                      qt_d[jd * 128:(jd + 1) * 128, c0:c0 + w])
                qgt = qg.tile([128, NFC * 512], bf, tag="qgt")
                for jf in range(NFC):
                    ps = chps.tile([128, 512], f32, tag="chain")
                    for jd in range(NDC):
                        nc.tensor.matmul(
                            ps[:, :w],
                            w_tr[:, jd * TF + jf * 128: jd * TF + (jf + 1) * 128],
                            qt_sb[:, jd * w:(jd + 1) * w],
                            start=(jd == 0), stop=(jd == NDC - 1))
                    nc.scalar.activation(qgt[:, jf * w:(jf + 1) * w], ps[:, :w],
                                         AF.Tanh, bias=b_tr[:, jf:jf + 1])
                qtt = qg.tile([128, NFC * 512], bf, tag="qtt")
                qwt = qg.tile([128, NFC * 512], bf, tag="qwt")
                for jz in range(NFC):
                    ps = chps.tile([128, 512], f32, tag="chain")
                    for jf in range(NFC):
                        nc.tensor.matmul(
                            ps[:, :w],
                            w_q[:, jf * TF + jz * 128: jf * TF + (jz + 1) * 128],
                            qgt[:, jf * w:(jf + 1) * w],
                            start=(jf == 0), stop=(jf == NFC - 1))
                    nc.scalar.add(qtt[:, jz * w:(jz + 1) * w], ps[:, :w], b_q[:, jz:jz + 1])
                    ps2 = chps.tile([128, 512], f32, tag="chain")
                    for jf in range(NFC):
                        nc.tensor.matmul(
                            ps2[:, :w],
                            w_W[:, jf * TF + jz * 128: jf * TF + (jz + 1) * 128],
                            qgt[:, jf * w:(jf + 1) * w],
                            start=(jf == 0), stop=(jf == NFC - 1))
                    nc.scalar.copy(qwt[:, jz * w:(jz + 1) * w], ps2[:, :w])

                for ci in range(w // 128):
                    col0 = ci * 128
                    den = accp.tile([128, B * NH], f32, tag="den")
                    rall = accp.tile([128, B * NH], f32, tag="rall")
                    for bb in range(B):
                        for z in range(NH):
                            jz, hz = z // 2, (z % 2) * 64
                            idx = bb * NH + z
                            ps_s = scps.tile([128, 512], f32, tag="ps_s")
                            nc.tensor.matmul(
                                ps_s[:],
                                qtt[hz:hz + 64, jz * w + col0: jz * w + col0 + 128],
                                wkt[hz:hz + 64, jz * BT + bb * 512: jz * BT + (bb + 1) * 512],
                                start=True, stop=True)
                            ps_g = gps.tile([128, 512], f32, tag="ps_g")
                            nc.tensor.matmul(
                                ps_g[:],
                                qwt[hz:hz + 64, jz * w + col0: jz * w + col0 + 128],
                                wvt[hz:hz + 64, jz * BT + bb * 512: jz * BT + (bb + 1) * 512],
                                start=True, stop=True)
                            et = expp.tile([128, 512], bf, tag="et")
                            sc = scr.tile([128, 512], bf, tag="sc")
                            if a_zero:
                                nc.scalar.activation(et[:], ps_s[:], AF.Exp,
                                                     accum_out=den[:, idx:idx + 1])
                                nc.vector.tensor_mul(sc[:], et[:], ps_g[:])
                                nc.vector.tensor_reduce(
                                    out=rall[:, idx:idx + 1], in_=sc[:],
                                    axis=mybir.AxisListType.X, op=ALU.add)
                            else:
                                nc.scalar.activation(et[:], ps_s[:], AF.Exp)
                                et2 = expp.tile([128, 512], bf, tag="et2")
                                nc.vector.tensor_mul(
                                    et2[:], et[:],
                                    ea_sb[:, bb * 512:(bb + 1) * 512])
                                nc.vector.tensor_reduce(
                                    out=den[:, idx:idx + 1], in_=et2[:],
                                    axis=mybir.AxisListType.X, op=ALU.add)
                                nc.vector.tensor_mul(sc[:], et2[:], ps_g[:])
                                nc.vector.tensor_reduce(
                                    out=rall[:, idx:idx + 1], in_=sc[:],
                                    axis=mybir.AxisListType.X, op=ALU.add)
                    rden = accp.tile([128, B * NH], f32, tag="rden")
                    nc.vector.reciprocal(rden[:], den[:])
                    rn = accp.tile([128, B * NH], f32, tag="rn")
                    nc.vector.tensor_tensor(out=rn[:], in0=rall[:], in1=rden[:],
                                            op=ALU.mult)
                    ot = outp.tile([128, B], f32, tag="ot")
                    nc.vector.tensor_reduce(
                        out=ot[:], in_=rn[:].rearrange("p (b z) -> p b z", z=NH),
                        axis=mybir.AxisListType.X, op=ALU.add)
                    nc.sync.dma_start(out_d[c0 + col0: c0 + col0 + 128, :], ot[:])

    nc.compile()
    return nc


def _get_nc(a_zero: bool):
    key = ("nc", a_zero)
    if key not in _CACHE:
        _CACHE[key] = _build(a_zero)
    return _CACHE[key]


def _prep_inputs(Q, H, a, trans_w, trans_b, q_w, q_b, k_w, k_b, v_w, v_b, W_w):
    """Host-side sharding/layout. Returns (in_maps, a_zero)."""
    a = np.asarray(a, np.float32)
    a_zero = not np.any(a)

    qt_full = np.zeros((D, CP), _BF)
    qt_full[:, :C_FULL] = np.asarray(Q, np.float32).T.astype(_BF)
    ht = np.ascontiguousarray(
        np.asarray(H, np.float32).reshape(BT, D).T.astype(_BF))
    shared = {
        "ht": ht,
        "wtr": np.ascontiguousarray(np.asarray(trans_w, np.float32).T.astype(_BF)),
        "wq": np.ascontiguousarray(np.asarray(q_w, np.float32).T.astype(_BF)),
        "wk": np.ascontiguousarray(np.asarray(k_w, np.float32).T.astype(_BF)),
        "wv": np.ascontiguousarray(np.asarray(v_w, np.float32).T.astype(_BF)),
        "ww": np.ascontiguousarray(np.asarray(W_w, np.float32).T.astype(_BF)),
        "btr": np.asarray(trans_b, np.float32),
        "bq": np.asarray(q_b, np.float32),
        "bk": np.asarray(k_b, np.float32),
        "bv": np.asarray(v_b, np.float32),
    }
    if not a_zero:
        ea = np.exp(a).reshape(1, BT).astype(_BF)
        shared["ea"] = np.ascontiguousarray(np.broadcast_to(ea, (128, BT)))
    in_maps = []
    for c in range(N_CORES):
        m = dict(shared)
        m["qt"] = np.ascontiguousarray(qt_full[:, c * CS:(c + 1) * CS])
        in_maps.append(m)
    return in_maps, a_zero


def kernel(**inputs) -> np.ndarray:
    from concourse.bass_utils import run_bass_kernel_spmd

    in_maps, a_zero = _prep_inputs(**inputs)
    nc = _get_nc(a_zero)
    res = run_bass_kernel_spmd(nc, in_maps, list(range(N_CORES)))
    out = np.concatenate([res.results[c]["out"] for c in range(N_CORES)], axis=0)
    return np.ascontiguousarray(out[:C_FULL, :].T)
